# revision 16
# baseline (speedup 1.0000x reference)
"""Causal self-attention (RoPE) Trainium2 Bass kernel.

Sharding: 8 cores = 4 batches x 2 head-groups. Core c handles batch c//2 and
heads (c%2)*8 .. (c%2)*8+7. Each core computes its QKV projection slice, RoPE,
causal flash-style attention in transposed layout, and a partial output
projection; the host sums the two partial projections per batch.

Projections run in float32r (TF32-like, full PE rate at 512-wide streams);
attention runs in bf16 which lifts the fp32r >=256-wide restriction so causal
tiles trim to their exact valid width. The whole kernel is one software-
pipelined loop over 512-column chunks: projections for chunk qc+1 are emitted
interleaved into the attention of chunk qc so the PE array never drains (and
stays at its top p-state) while the Activation engine works through the
softmax exponentials.

Softmax normalization: V for each head pair is stored as
[V_A(64) | ones | ones | V_B(64)] so the A-half AV matmul lands at PSUM
partitions 0:65 (y_A rows 0:64, denom_A row 64) and the B-half at partitions
63:128 (denom_B row 63, y_B rows 64:128). Both denominators are reciprocal'd
in-lane by the DVE, broadcast with a single precomputed selector matmul to a
[128,512] tile, and the final scale-multiplies are lane-aligned DVE ops (no
cross-partition traffic, no SBUF-to-SBUF DMA).

Engine balance: PSUM evictions (v, out) and the RoPE sin-multiply run on the
otherwise idle GPSIMD engine, keeping the DVE for masks/rope-add/norm and the
Activation engine exclusively on exponentials.
"""

import math
import numpy as np
from contextlib import ExitStack

import ml_dtypes
import concourse.bass as bass
import concourse.tile as tile
from concourse import bacc, mybir
from concourse.bass_utils import run_bass_kernel_spmd

F32 = mybir.dt.float32
R32 = mybir.dt.float32r
BF16 = mybir.dt.bfloat16
EXPF = mybir.ActivationFunctionType.Exp
MULT = mybir.AluOpType.mult
ADD = mybir.AluOpType.add
BF = ml_dtypes.bfloat16

B, T, C, H, D = 4, 2048, 1024, 16, 64
HL = 8            # local heads per core
NP = HL // 2      # head pairs per core
KT = C // 128     # contraction tiles for projections
TT = T // 128     # 128-row tiles of T
QC = T // 512     # 512-col chunks of T
SCALE = 1.0 / math.sqrt(D)

_CACHE = {}


def _build_nc():
    nc = bacc.Bacc("TRN2", debug=False, num_devices=8)

    xT_d = nc.dram_tensor("xT", [KT, 128, T], R32, kind="ExternalInput").ap()
    wq_d = nc.dram_tensor("wq", [128, NP, KT, 128], R32, kind="ExternalInput").ap()
    wk_d = nc.dram_tensor("wk", [128, NP, KT, 128], R32, kind="ExternalInput").ap()
    wv_d = nc.dram_tensor("wv", [128, KT, 512], R32, kind="ExternalInput").ap()
    wo_d = nc.dram_tensor("wo", [128, NP, C], BF16, kind="ExternalInput").ap()
    cos_d = nc.dram_tensor("cosT", [128, T], F32, kind="ExternalInput").ap()
    sin_d = nc.dram_tensor("sinT", [128, T], F32, kind="ExternalInput").ap()
    psw_d = nc.dram_tensor("psw", [128, 128], BF16, kind="ExternalInput").ap()
    e128_d = nc.dram_tensor("e128", [128, 128], BF16, kind="ExternalInput").ap()
    msk_d = nc.dram_tensor("msk", [128, 4, 512], BF16, kind="ExternalInput").ap()
    out_d = nc.dram_tensor("out", [T, C], F32, kind="ExternalOutput").ap()

    with tile.TileContext(nc) as tc:
        with ExitStack() as ctx:
            pers = ctx.enter_context(tc.tile_pool(name="pers", bufs=1))
            qkT = {}
            for p in range(NP):
                for s in "qk":
                    qkT[(p, s)] = pers.tile([128, T], BF16, name=f"qkT_{p}_{s}")
            yT = [pers.tile([128, T], BF16, name=f"yT_{r}") for r in range(NP)]
            vext = pers.tile([128, TT, NP, 193], BF16)
            wq_sb = pers.tile([128, NP, KT, 128], R32)
            wk_sb = pers.tile([128, NP, KT, 128], R32)
            wv_sb = pers.tile([128, KT, 512], R32)
            wo_sb = pers.tile([128, NP, C], BF16)
            cos_sb = pers.tile([128, T], F32)
            sin_sb = pers.tile([128, T], F32)
            psw_sb = pers.tile([128, 128], BF16)
            e128_sb = pers.tile([128, 128], BF16)
            msk_sb = pers.tile([128, 4, 512], BF16)
            rrec = pers.tile([128, 512], BF16)

            xp = ctx.enter_context(tc.tile_pool(name="xp", bufs=2))
            xc_tiles = {}

            def emit_xdma(qc):
                xc = xp.tile([128, KT, 512], R32, tag="xc", name=f"xc{qc}")
                for kt in range(KT):
                    nc.sync.dma_start(
                        xc[:, kt], xT_d[kt, :, qc * 512 : (qc + 1) * 512]
                    )
                xc_tiles[qc] = xc

            # first-needed data first: chunk-0 x and wv feed the very first
            # matmul; wq/wk + rope tables next; wo is not needed until the
            # first output projection a quarter of the way in.
            xc0 = xp.tile([128, KT, 512], R32, tag="xc", name="xc0")
            for kt in range(KT):
                nc.sync.dma_start(xc0[:, kt], xT_d[kt, :, 0:512])
                nc.sync.dma_start(wv_sb[:, kt], wv_d[:, kt])
            xc_tiles[0] = xc0
            nc.sync.dma_start(wq_sb[:, 0], wq_d[:, 0])
            nc.sync.dma_start(wk_sb[:, 0], wk_d[:, 0])
            nc.sync.dma_start(cos_sb[:, 0:512], cos_d[:, 0:512])
            nc.sync.dma_start(sin_sb[:, 0:512], sin_d[:, 0:512])
            nc.sync.dma_start(psw_sb[:], psw_d)
            for pair in range(1, NP):
                nc.sync.dma_start(wq_sb[:, pair], wq_d[:, pair])
                nc.sync.dma_start(wk_sb[:, pair], wk_d[:, pair])
            nc.sync.dma_start(cos_sb[:, 512:T], cos_d[:, 512:T])
            nc.sync.dma_start(sin_sb[:, 512:T], sin_d[:, 512:T])
            nc.sync.dma_start(e128_sb[:], e128_d)
            nc.sync.dma_start(msk_sb[:], msk_d)
            nc.sync.dma_start(wo_sb[:], wo_d)
            nc.gpsimd.memset(vext[:, :, :, 64:65], 1.0)
            nc.gpsimd.memset(vext[:, :, :, 97:98], 1.0)
            nc.gpsimd.memset(vext[:, :, :, 65:97], 0.0)
            nc.gpsimd.memset(vext[:, :, :, 98:129], 0.0)
            nc.gpsimd.memset(rrec[:], 0.0)

            with (
                tc.tile_pool(name="sbp", bufs=2) as sbp,
                tc.tile_pool(name="mmp", bufs=2, space="PSUM") as mmp,
                tc.tile_pool(name="scp", bufs=2, space="PSUM") as scp,
                tc.tile_pool(name="pyp", bufs=1, space="PSUM") as pyp,
            ):
                def emit_v2(qc, half):
                    xc = xc_tiles[qc]
                    for i in (2 * half, 2 * half + 1):
                        tt = 4 * qc + i
                        ps = mmp.tile([128, 512], F32, tag="mm", name=f"psv{tt}")
                        for kt in range(KT):
                            nc.tensor.matmul(
                                ps[:],
                                xc[:, kt, i * 128 : (i + 1) * 128],
                                wv_sb[:, kt],
                                start=(kt == 0),
                                stop=(kt == KT - 1),
                            )
                        # scatter [tok, (h,d)] into the padded per-pair vext
                        # layout on the gpsimd engine (the DVE stays on the
                        # softmax path)
                        pv = ps[:].rearrange("c (p x) -> c p x", p=NP)
                        nc.vector.tensor_copy(
                            vext[:, tt, :, 0:64], pv[:, :, 0:64]
                        )
                        nc.vector.tensor_copy(
                            vext[:, tt, :, 129:193], pv[:, :, 64:128]
                        )

                def emit_qk(qc, pair):
                    lo = qc * 512
                    xc = xc_tiles[qc]
                    pss = {}
                    for s, w_sb in (("q", wq_sb), ("k", wk_sb)):
                        ps = mmp.tile([128, 512], F32, tag="mm", name=f"ps{s}")
                        for kt in range(KT):
                            nc.tensor.matmul(
                                ps[:], w_sb[:, pair, kt], xc[:, kt],
                                start=(kt == 0), stop=(kt == KT - 1),
                            )
                        pss[s] = ps
                    prs = {}
                    for s in "qk":
                        ps = pss[s]
                        dst = qkT[(pair, s)][:, lo : lo + 512]
                        u = sbp.tile([128, 512], BF16, tag="u", name="u")
                        nc.vector.tensor_tensor(
                            u[:], ps[:], sin_sb[:, lo : lo + 512], MULT
                        )
                        nc.vector.tensor_tensor(
                            dst, ps[:], cos_sb[:, lo : lo + 512], MULT
                        )
                        pr = mmp.tile([128, 512], F32, tag="mm", name="pr")
                        nc.tensor.matmul(pr[:], psw_sb[:], u[:], start=True, stop=True)
                        prs[s] = pr
                    for s in "qk":
                        dst = qkT[(pair, s)][:, lo : lo + 512]
                        nc.vector.tensor_tensor(dst, prs[s][:], dst, ADD)

                def emit_att_kts(qc, p, psyA, psyB):
                    lo, hi = qc * 512, (qc + 1) * 512
                    nkt = (qc + 1) * 4
                    qTt = qkT[(p, "q")]
                    kTt = qkT[(p, "k")]

                    def emit_sc(kt):
                        klo, khi = kt * 128, (kt + 1) * 128
                        off = klo - lo
                        tr = off if off > 0 else 0
                        ps2 = scp.tile([128, 1024], F32, tag="sc", name="ps2")
                        p3 = ps2[:].rearrange("p (h n) -> p h n", h=2)
                        nc.tensor.matmul(
                            ps2[:, tr:512],
                            kTt[0:64, klo:khi], qTt[0:64, lo + tr : hi],
                            start=True, stop=True,
                        )
                        nc.tensor.matmul(
                            ps2[:, 512 + tr : 1024],
                            kTt[64:128, klo:khi], qTt[64:128, lo + tr : hi],
                            start=True, stop=True,
                        )
                        aAB = sbp.tile(
                            [128, 1024], BF16, tag="a", bufs=4, name="aAB"
                        )
                        a3 = aAB[:].rearrange("p (h n) -> p h n", h=2)
                        nc.scalar.activation(
                            a3[:, :, tr:512], p3[:, :, tr:512], EXPF, scale=SCALE
                        )
                        if off >= 0:
                            mi = off // 128
                            for h in range(2):
                                sl = a3[:, h, tr:512]
                                nc.vector.tensor_tensor(
                                    sl, sl, msk_sb[:, mi, tr:512], MULT
                                )
                        return aAB, tr

                    def emit_av(kt, aAB, tr):
                        first, last = kt == 0, kt == nkt - 1
                        nc.tensor.matmul(
                            psyA[:, tr:512], vext[:, kt, p, 0:65],
                            aAB[:, tr:512], start=first, stop=last,
                        )
                        nc.tensor.matmul(
                            psyB[:, tr:512], vext[:, kt, p, 65:193],
                            aAB[:, 512 + tr : 1024], start=first, stop=last,
                        )

                    prev = None
                    for kt in range(nkt):
                        cur = emit_sc(kt)
                        if prev is not None:
                            emit_av(kt - 1, *prev)
                        prev = cur
                    emit_av(nkt - 1, *prev)

                def emit_norm_a(psyA, psyB):
                    # denominator rows to SBUF (scalar engine); the broadcast
                    # matmul that consumes them is emitted after a whole
                    # projection chunk, so the PE never waits on this
                    nc.scalar.copy(rrec[64:65, :], psyA[64:65, :])
                    nc.scalar.copy(rrec[32:33, :], psyB[32:33, :])

                def emit_norm_b(qc, p, psyA, psyB):
                    lo, hi = qc * 512, (qc + 1) * 512
                    pbc = mmp.tile([128, 512], F32, tag="mm", name="pbc")
                    nc.tensor.matmul(
                        pbc[:], e128_sb[:], rrec[:], start=True, stop=True
                    )
                    br = sbp.tile([128, 512], F32, tag="br", name="br")
                    nc.vector.reciprocal_approx_fast(br[:], pbc[:])
                    nc.vector.tensor_tensor(
                        yT[p][0:64, lo:hi], psyA[0:64, :], br[0:64, :], MULT
                    )
                    nc.vector.tensor_tensor(
                        yT[p][64:128, lo:hi], psyB[64:128, :], br[64:128, :], MULT
                    )

                def emit_out(qc):
                    for i in range(4):
                        mlo = (4 * qc + i) * 128
                        for cc in range(2):
                            clo = cc * 512
                            ps = mmp.tile([128, 512], F32, tag="mm", name="pso")
                            for r in range(NP):
                                nc.tensor.matmul(
                                    ps[:],
                                    yT[r][:, mlo : mlo + 128],
                                    wo_sb[:, r, clo : clo + 512],
                                    start=(r == 0), stop=(r == NP - 1),
                                )
                            ob = sbp.tile([128, 512], F32, tag="ob", name="ob")
                            nc.vector.tensor_copy(ob[:], ps[:])
                            nc.sync.dma_start(
                                out_d[mlo : mlo + 128, clo : clo + 512], ob[:]
                            )

                # prologue: chunk 0 projections
                emit_v2(0, 0)
                emit_v2(0, 1)
                for pair in range(NP):
                    emit_qk(0, pair)

                pending_out = [None]

                for qc in range(QC):
                    nqc = qc + 1
                    has_next = nqc < QC
                    if has_next:
                        emit_xdma(nqc)
                    # per-pair: attention kts, then a projection chunk for the
                    # next qc (fills PE while DVE/Act work), then normalization
                    items = (
                        [
                            lambda: emit_v2(nqc, 0),
                            lambda: emit_v2(nqc, 1),
                            lambda: emit_qk(nqc, 0),
                            lambda: emit_qk(nqc, 1),
                        ]
                        if has_next
                        else [None] * 4
                    )
                    for p in range(NP):
                        psyA = pyp.tile([65, 512], F32, tag="pyA", name="psyA")
                        psyB = pyp.tile([128, 512], F32, tag="pyB", name="psyB")
                        emit_att_kts(qc, p, psyA, psyB)
                        emit_norm_a(psyA, psyB)
                        if p == 0 and pending_out[0] is not None:
                            # previous chunk's output projection: PE filler
                            # during this chunk's ACT-bound attention, with
                            # the DVE free for its PSUM evictions
                            emit_out(pending_out[0])
                        if items[p] is not None:
                            items[p]()
                        emit_norm_b(qc, p, psyA, psyB)
                    if has_next:
                        emit_qk(nqc, 2)
                        emit_qk(nqc, 3)
                    pending_out[0] = qc
                emit_out(QC - 1)

    nc.compile()
    return nc


def _host_tables():
    half = D // 2
    freq = np.exp(-math.log(10000.0) * np.arange(half) / half).astype(np.float64)
    ang = np.arange(T, dtype=np.float64)[None, :] * freq[:, None]  # [32, T]
    cos32 = np.cos(ang).astype(np.float32)
    sin32 = np.sin(ang).astype(np.float32)
    cosT = np.tile(cos32, (4, 1))                                   # [128, T]
    sinT = np.concatenate([sin32, -sin32, sin32, -sin32], axis=0)   # [128, T]
    psw = np.zeros((128, 128), np.float32)
    psw[np.arange(128) ^ 32, np.arange(128)] = 1.0
    e128 = np.zeros((128, 128), np.float32)
    e128[64, 0:64] = 1.0
    e128[32, 64:128] = 1.0
    kk = np.arange(128)[:, None, None]
    ii = np.arange(4)[None, :, None]
    qq = np.arange(512)[None, None, :]
    msk = (qq >= kk + ii * 128).astype(np.float32)
    return cosT, sinT, psw.astype(BF), e128.astype(BF), msk.astype(BF)


def _pack_weights(w_qkv, w_out, hg):
    lo, hi = hg * HL, (hg + 1) * HL
    wqf = w_qkv[:, 0:C].reshape(C, H, D)[:, lo:hi]       # [C, 8, D]
    wkf = w_qkv[:, C : 2 * C].reshape(C, H, D)[:, lo:hi]
    wvf = w_qkv[:, 2 * C : 3 * C].reshape(C, H, D)[:, lo:hi]

    def pack_qk(w):
        a = w.reshape(KT, 128, NP, 2, D)
        return np.ascontiguousarray(
            a.transpose(1, 2, 0, 3, 4).reshape(128, NP, KT, 128)
        )

    wq = pack_qk(wqf)
    wk = pack_qk(wkf)
    wv = np.ascontiguousarray(
        wvf.reshape(KT, 128, HL * D).transpose(1, 0, 2)
    )
    wo_l = w_out.reshape(H, D, C)[lo:hi].reshape(NP, 128, C)
    wo = np.ascontiguousarray(wo_l.transpose(1, 0, 2)).astype(BF)
    return wq, wk, wv, wo


def _in_maps(x, w_qkv, w_out):
    x = np.asarray(x, dtype=np.float32)
    w_qkv = np.asarray(w_qkv, dtype=np.float32)
    w_out = np.asarray(w_out, dtype=np.float32)
    cosT, sinT, psw, e128, msk = _host_tables()
    packs = [_pack_weights(w_qkv, w_out, hg) for hg in range(2)]
    xTs = [
        np.ascontiguousarray(x[b].T).reshape(KT, 128, T) for b in range(B)
    ]
    in_maps = []
    for c in range(8):
        b, hg = c // 2, c % 2
        wq, wk, wv, wo = packs[hg]
        in_maps.append(
            {
                "xT": xTs[b], "wq": wq, "wk": wk, "wv": wv, "wo": wo,
                "cosT": cosT, "sinT": sinT, "psw": psw, "e128": e128,
                "msk": msk,
            }
        )
    return in_maps


def kernel(x, w_qkv, w_out):
    if "nc" not in _CACHE:
        _CACHE["nc"] = _build_nc()
    nc = _CACHE["nc"]

    in_maps = _in_maps(x, w_qkv, w_out)
    res = run_bass_kernel_spmd(nc, in_maps, core_ids=list(range(8)))
    outs = [np.asarray(res.results[c]["out"], np.float32) for c in range(8)]
    y = np.stack([outs[2 * b] + outs[2 * b + 1] for b in range(B)], axis=0)
    return y.astype(np.float32)
